# revision 8
# baseline (speedup 1.0000x reference)
"""Multi-head causal attention (B=2, S=2048, C=2048, H=16) on 8 NeuronCores.

Sharding: 2-way data parallel over batch x 4-way tensor parallel over heads.
Core i handles batch b = i // 4 and heads [4*(i%4), 4*(i%4)+4).

v3 kernel design (s-major scores, no denominator matmul):
  phase A: Q/K (d on partitions) and V (s on partitions) projections in f16,
           weights SBUF-cached via bulk DMAs on the gpsimd queue, c-inner
           accumulation (one PSUM bank per output group, N=512 streams),
           Q/K evacuated on the scalar engine (Identity + bias AP, SCALE
           folded into Q), V on DVE.
  phase B: scores s-major (stationary Q chunk [d,128], moving K) into
           [128,1024] two-bank PSUM pairs; causal mask on the diagonal
           128-block via DVE add of a -100 tile; exp on ACT with accum_out
           (softmax denominator for free, one op per 1024 cols); reciprocal
           appended as an extra f16 column; xbar DMA transpose flips
           [s,t] -> [t,s] blocks incl. the rec column (transposes are
           emitted one pipeline unit late so they never stall the sync
           queue); AV streams the transposed probabilities (N=512) against
           stationary V; the reciprocal row is partition-broadcast (gpsimd)
           and multiplied in during PSUM evacuation (DVE).
  phase C: local output projection partials, evacuated on ACT; bo is added
           on the host during the cross-core partial reduction.
  Schedule: phase B/C units of block sb-1 are interleaved 1:1 into phase A
           groups of block sb; for sb=3 the B/C units interleave with
           phase A of sb=3 itself so the tail stays dense.
"""

import numpy as np

B, S, C, H = 2, 2048, 2048, 16
D = C // H            # 128 per-head dim
HL = 4                # heads per core
ML = HL * D           # 512 local channels
P = 128
NCT = C // P          # 16 contraction tiles
NT = S // P           # 16 key tiles
SCALE = 1.0 / float(np.sqrt(D))

_CACHE = {}


def _build():
    import concourse.bacc as bacc
    import concourse.mybir as mybir
    import concourse.tile as tile

    f32 = mybir.dt.float32
    f16 = mybir.dt.float16
    Exp = mybir.ActivationFunctionType.Exp
    Ident = mybir.ActivationFunctionType.Identity
    Copy = mybir.ActivationFunctionType.Copy
    add = mybir.AluOpType.add
    X = mybir.AxisListType.X

    nc = bacc.Bacc("TRN2", target_bir_lowering=False, debug=False, num_devices=8)

    xt = nc.dram_tensor("xt", [C, S], f16, kind="ExternalInput")       # x[b].T
    wqt = nc.dram_tensor("wqt", [C, ML], f16, kind="ExternalInput")    # Wq.T cols
    wkt = nc.dram_tensor("wkt", [C, ML], f16, kind="ExternalInput")
    wvt = nc.dram_tensor("wvt", [C, ML], f16, kind="ExternalInput")
    wot = nc.dram_tensor("wot", [ML, C], f16, kind="ExternalInput")    # Wo.T rows
    bqs = nc.dram_tensor("bqs", [ML], f32, kind="ExternalInput")       # bq * SCALE
    bk = nc.dram_tensor("bk", [ML], f32, kind="ExternalInput")
    bv = nc.dram_tensor("bv", [ML], f16, kind="ExternalInput")
    maskd = nc.dram_tensor("maskd", [P, P], f32, kind="ExternalInput") # 0 / -100
    out = nc.dram_tensor("out", [S, C], f16, kind="ExternalOutput")

    xt_r = xt[:, :].rearrange("(c p) s -> p c s", p=P)    # [128, 16, 2048]
    wq_r = wqt[:, :].rearrange("(c p) m -> p c m", p=P)   # [128, 16, 512]
    wk_r = wkt[:, :].rearrange("(c p) m -> p c m", p=P)
    wv_r = wvt[:, :].rearrange("(c p) m -> p c m", p=P)
    wo_r = wot[:, :].rearrange("(m p) j -> p m j", p=P)   # [128, 4, 2048]
    out_r = out[:, :].rearrange("(g st p) j -> g p st j", p=P, st=4)  # [4, 128, 4, 2048]

    with tile.TileContext(nc) as tc:
        with tc.tile_pool(name="persist", bufs=1) as pp_, \
             tc.tile_pool(name="work", bufs=1) as wk, \
             tc.tile_pool(name="psp", bufs=1, space="PSUM") as psp:

            wvc = pp_.tile([P, NCT, ML], f16, tag="wvc", name="wvc")
            wqc = pp_.tile([P, NCT, ML], f16, tag="wqc", name="wqc")
            wkc = pp_.tile([P, NCT, ML], f16, tag="wkc", name="wkc")
            woc = pp_.tile([P, HL, C], f16, tag="woc", name="woc")
            K = [pp_.tile([P, S], f16, tag=f"k{m}", name=f"k{m}") for m in range(HL)]
            V = [pp_.tile([P, ML], f16, tag=f"v{t}", name=f"v{t}") for t in range(NT)]
            maskT = pp_.tile([P, P], f32, tag="maskT", name="maskT")
            bv_row = pp_.tile([1, ML], f16, tag="bv_row", name="bv_row")
            bv_bc = pp_.tile([P, ML], f16, tag="bv_bc", name="bv_bc")
            bqs_t = [pp_.tile([P, 1], f32, tag=f"bq{m}", name=f"bq{m}") for m in range(HL)]
            bk_t = [pp_.tile([P, 1], f32, tag=f"bk{m}", name=f"bk{m}") for m in range(HL)]

            # small loads on gpsimd; bulk loads on the fast sync HWDGE path
            nc.gpsimd.dma_start(maskT[:], maskd[:, :])
            nc.gpsimd.dma_start(bv_row[:], bv[None, :])
            for m in range(HL):
                nc.gpsimd.dma_start(bqs_t[m][:], bqs[m * P:(m + 1) * P, None])
                nc.gpsimd.dma_start(bk_t[m][:], bk[m * P:(m + 1) * P, None])
            nc.gpsimd.partition_broadcast(bv_bc[:], bv_row[:])

            # xt cache: contraction-split halves (c 0..7 / 8..15) per s-block
            xtc_tiles = {}

            def load_xtc(sb, ch, quarters=False):
                tag = "xtcA" if ch == 0 else "xtcB"
                bufs = 2 if ch == 0 else 1
                t = wk.tile([P, 8, 512], f16, tag=tag, bufs=bufs,
                            name=f"xtc{sb}{ch}")
                xtc_tiles[(sb, ch)] = t
                s0 = sb * 512
                if quarters:
                    for q in range(2):
                        nc.sync.dma_start(
                            t[:, q * 4:(q + 1) * 4, :],
                            xt_r[:, ch * 8 + q * 4:ch * 8 + (q + 1) * 4,
                                 s0:s0 + 512])
                else:
                    nc.sync.dma_start(t[:], xt_r[:, ch * 8:(ch + 1) * 8,
                                                  s0:s0 + 512])
                return t

            # startup: wv/xt quarters interleaved in first-use order
            for q in range(2):
                nc.sync.dma_start(wvc[:, q * 4:(q + 1) * 4, :],
                                  wv_r[:, q * 4:(q + 1) * 4, :])
                if q == 0:
                    load_xtc(0, 0, quarters=True)
            for q in range(2, 4):
                nc.sync.dma_start(wvc[:, q * 4:(q + 1) * 4, :],
                                  wv_r[:, q * 4:(q + 1) * 4, :])
                if q == 2:
                    load_xtc(0, 1)
            nc.sync.dma_start(wqc[:], wq_r)
            nc.sync.dma_start(wkc[:], wk_r)
            nc.sync.dma_start(woc[:], wo_r)

            state = {}

            # ---------------- phase A groups ----------------
            def a_group(sb, kind, idx, last_group=False):
                def go():
                    xa = xtc_tiles[(sb, 0)]
                    xb = xtc_tiles[(sb, 1)]
                    order = list(range(16))
                    if last_group:
                        order = list(range(8, 16)) + list(range(8))
                    acc = psp.tile([P, 512], f32, tag="pa", bufs=2,
                                   name=f"pa_{sb}{kind}{idx}")
                    for n, c in enumerate(order):
                        ch, c8 = divmod(c, 8)
                        xtile = xa if ch == 0 else xb
                        if kind == "v":
                            nc.tensor.matmul(acc[:],
                                             xtile[:, c8, idx * P:(idx + 1) * P],
                                             wvc[:, c, :],
                                             start=(n == 0), stop=(n == 15))
                        else:
                            wc = wqc if kind == "q" else wkc
                            nc.tensor.matmul(acc[:],
                                             wc[:, c, idx * P:(idx + 1) * P],
                                             xtile[:, c8, :],
                                             start=(n == 0), stop=(n == 15))
                    if kind == "v":
                        nc.vector.tensor_add(V[sb * 4 + idx][:], acc[:], bv_bc[:])
                    elif kind == "q":
                        qsb = state[("q", sb)]
                        nc.scalar.activation(qsb[idx][:], acc[:], Ident,
                                             bias=bqs_t[idx][:], scale=SCALE)
                    else:
                        s0 = sb * 512
                        nc.scalar.activation(K[idx][:, s0:s0 + 512], acc[:],
                                             Ident, bias=bk_t[idx][:])
                return go

            # ---------------- phase B/C units ----------------
            def b_scores(sb, h):
                """scores + exp + denominator for head h (no transpose)."""
                def go():
                    s0 = sb * 512
                    qsb = state[("q", sb)]
                    for sc in range(4):
                        t_end = s0 + sc * P + P
                        ntt = (t_end + 511) // 512
                        pes = wk.tile([P, (14 + sc) * P], f16, tag=f"pes{sc}",
                                      bufs=2, name=f"pes{sb}{h}{sc}")
                        den4 = wk.tile([P, 2], f32, tag=f"den{sc}", bufs=2,
                                       name=f"den{sb}{h}{sc}")
                        rec1 = wk.tile([P, 1], f32, tag=f"rec{sc}", bufs=2,
                                       name=f"rec{sb}{h}{sc}")
                        ps = None
                        for tt in range(ntt):
                            w = min(512, t_end - tt * 512)
                            tp, off = divmod(tt, 2)
                            if off == 0:
                                ps = psp.tile([P, 1024], f32, tag="ps", bufs=2,
                                              name=f"ps{sb}{h}{sc}{tp}")
                            nc.tensor.matmul(ps[:, off * 512:off * 512 + w],
                                             qsb[h][:, sc * P:(sc + 1) * P],
                                             K[h][:, tt * 512:tt * 512 + w],
                                             start=True, stop=True)
                            if tt == ntt - 1:
                                w2 = off * 512 + w
                                nc.vector.tensor_add(ps[:, w2 - P:w2],
                                                     ps[:, w2 - P:w2], maskT[:])
                            if off == 1 or tt == ntt - 1:
                                w2 = off * 512 + w
                                nc.scalar.activation(
                                    pes[:, tp * 1024:tp * 1024 + w2],
                                    ps[:, :w2], Exp,
                                    accum_out=den4[:, tp:tp + 1])
                        ntp = (ntt + 1) // 2
                        if ntp > 1:
                            dsum = wk.tile([P, 1], f32, tag=f"dsum{sc}", bufs=2,
                                           name=f"dsum{sb}{h}{sc}")
                            nc.vector.tensor_reduce(dsum[:], den4[:, :ntp], X, add)
                            nc.vector.reciprocal(rec1[:], dsum[:])
                        else:
                            nc.vector.reciprocal(rec1[:], den4[:, 0:1])
                        nc.vector.tensor_copy(pes[:, t_end:t_end + 1], rec1[:])
                        state[("pes", sb, h, sc)] = pes
                return go

            def b_transpose(sb, h):
                """xbar transposes for head h (emitted one unit late)."""
                def go():
                    s0 = sb * 512
                    pet = wk.tile([P, 17, 512], f16, tag="pet", bufs=2,
                                  name=f"pet{sb}{h}")
                    state[("pet", sb, h)] = pet
                    for sc in range(4):
                        t_end = s0 + sc * P + P
                        nblk = t_end // P + 1
                        pes = state[("pes", sb, h, sc)]
                        nc.sync.dma_start_transpose(
                            pet[:, :nblk, sc * P:(sc + 1) * P],
                            pes[:, :nblk * P])
                return go

            def b_av(sb, h):
                def go():
                    s0 = sb * 512
                    pet = state[("pet", sb, h)]
                    recbc = wk.tile([P, 512], f16, tag="recbc", bufs=2,
                                    name=f"recbc{sb}{h}")
                    for sc in range(4):
                        nblk = (s0 + sc * P + P) // P + 1
                        nc.gpsimd.partition_broadcast(
                            recbc[:, sc * P:(sc + 1) * P],
                            pet[0:1, nblk - 1, sc * P:(sc + 1) * P])
                    ntile = 4 * sb + 4
                    po = psp.tile([P, 512], f32, tag="acc", bufs=2,
                                  name=f"po{sb}{h}")
                    for ti in range(ntile):
                        jj = max(0, ti - 4 * sb)
                        nc.tensor.matmul(po[:, jj * P:512],
                                         V[ti][:, h * P:(h + 1) * P],
                                         pet[:, ti, jj * P:512],
                                         start=(ti == 0), stop=(ti == ntile - 1))
                    oth = wk.tile([P, 512], f16, tag=f"ot{h}", bufs=1,
                                  name=f"ot{sb}{h}")
                    state[("ot", sb)][h] = oth
                    nc.vector.tensor_mul(oth[:], po[:], recbc[:])
                return go

            def c_unit(sb, jb):
                def go():
                    j0 = jb * 512
                    ot = state[("ot", sb)]
                    outt = wk.tile([P, 4, 512], f16, tag="outt", bufs=1,
                                   name=f"outt{sb}{jb}")
                    for st in range(4):
                        ppt = psp.tile([P, 512], f32, tag="acc", bufs=2,
                                       name=f"pp{sb}{jb}{st}")
                        for m in range(HL):
                            nc.tensor.matmul(ppt[:],
                                             ot[m][:, st * P:(st + 1) * P],
                                             woc[:, m, j0:j0 + 512],
                                             start=(m == 0), stop=(m == HL - 1))
                        nc.scalar.activation(outt[:, st, :], ppt[:], Copy)
                    nc.sync.dma_start(out_r[sb, :, :, j0:j0 + 512], outt[:])
                return go

            def units_for(sb):
                def pair(*fs):
                    def go():
                        for f in fs:
                            f()
                    return go
                return [
                    b_scores(sb, 0),
                    pair(b_scores(sb, 1), b_transpose(sb, 0)),
                    pair(b_scores(sb, 2), b_transpose(sb, 1)),
                    pair(b_scores(sb, 3), b_transpose(sb, 2)),
                    b_av(sb, 0),
                    b_transpose(sb, 3),
                    b_av(sb, 1),
                    b_av(sb, 2),
                    b_av(sb, 3),
                    c_unit(sb, 0), c_unit(sb, 1), c_unit(sb, 2), c_unit(sb, 3),
                ]

            # ---------------- schedule ----------------
            # Half-window-shifted pipeline: each block's scores units drain
            # within its own window (gated on the Q/K groups they need);
            # AV/C units flow into the next window's V/Q groups.
            from collections import deque
            pending = deque()   # (gate_sb, gate_gj, closure)

            for sb in range(4):
                state[("q", sb)] = [wk.tile([P, 512], f16, tag=f"q{m}", bufs=2,
                                            name=f"q{m}_{sb}")
                                    for m in range(HL)]
                state[("ot", sb)] = [None] * HL

                groups = ([a_group(sb, "v", i) for i in range(4)] +
                          [a_group(sb, "q", 0), a_group(sb, "k", 0),
                           a_group(sb, "q", 1), a_group(sb, "k", 1),
                           a_group(sb, "q", 2), a_group(sb, "k", 2),
                           a_group(sb, "q", 3),
                           a_group(sb, "k", 3, last_group=True)])

                units = units_for(sb)
                gates = [6, 8, 10, 11, -1, -1, -1, -1, -1, -1, -1, -1, -1]
                for u, gate in zip(units, gates):
                    pending.append((sb, gate, u))

                popped = 0
                for gj, g in enumerate(groups):
                    if sb < 3 and gj == 10:
                        load_xtc(sb + 1, 0)
                    g()
                    if sb < 3 and gj == 11:
                        # after the last group: its reversed c-order frees
                        # the ch=1 buffer early for this overwrite
                        load_xtc(sb + 1, 1)

                    def eligible():
                        if not pending:
                            return False
                        gsb, ggj, _ = pending[0]
                        return gsb < sb or ggj <= gj
                    budget = (gj + 1) * 14 // 12
                    while eligible() and popped < budget:
                        pending.popleft()[2]()
                        popped += 1
            while pending:
                pending.popleft()[2]()

    nc.compile()
    return nc


def _get_program():
    if "nc" not in _CACHE:
        _CACHE["nc"] = _build()
    return _CACHE["nc"]


def make_in_maps(x, Wq, bq, Wk, bk, Wv, bv, Wo, bo):
    xtb = [np.ascontiguousarray(x[b].T).astype(np.float16) for b in range(B)]
    WqT = np.ascontiguousarray(Wq.T).astype(np.float16)
    WkT = np.ascontiguousarray(Wk.T).astype(np.float16)
    WvT = np.ascontiguousarray(Wv.T).astype(np.float16)
    WoT = np.ascontiguousarray(Wo.T).astype(np.float16)
    maskd = np.where(np.triu(np.ones((P, P), dtype=bool), k=1),
                     np.float32(-100.0), np.float32(0.0))
    in_maps = []
    for core in range(8):
        b, hg = divmod(core, 4)
        ms = slice(hg * ML, (hg + 1) * ML)
        in_maps.append({
            "xt": xtb[b],
            "wqt": np.ascontiguousarray(WqT[:, ms]),
            "wkt": np.ascontiguousarray(WkT[:, ms]),
            "wvt": np.ascontiguousarray(WvT[:, ms]),
            "wot": np.ascontiguousarray(WoT[ms, :]),
            "bqs": np.ascontiguousarray(bq[ms] * SCALE).astype(np.float32),
            "bk": np.ascontiguousarray(bk[ms]).astype(np.float32),
            "bv": np.ascontiguousarray(bv[ms]).astype(np.float16),
            "maskd": maskd,
        })
    return in_maps


def run(inputs, trace=False):
    from concourse.bass_utils import run_bass_kernel_spmd

    nc = _get_program()
    in_maps = make_in_maps(
        inputs["x"], inputs["Wq"], inputs["bq"], inputs["Wk"], inputs["bk"],
        inputs["Wv"], inputs["bv"], inputs["Wo"], inputs["bo"])
    res = run_bass_kernel_spmd(nc, in_maps, core_ids=list(range(8)), trace=trace)
    partials = [np.asarray(res.results[c]["out"]).astype(np.float32)
                for c in range(8)]
    bo64 = np.asarray(inputs["bo"], dtype=np.float64)
    full = np.empty((B, S, C), dtype=np.float32)
    for b in range(B):
        acc = np.sum(np.stack(partials[4 * b:4 * b + 4], 0), 0,
                     dtype=np.float64) + bo64
        full[b] = acc.astype(np.float32)
    return full, res


def kernel(**inputs):
    full, _ = run(inputs, trace=False)
    return full
